# revision 4
# baseline (speedup 1.0000x reference)
"""Trainium2 Bass kernel for the bilevel logit-assignment flow problem.

Reference math (N=384, cutoff-2 paths):
    A = (adj > 0) & ~eye
    E = A * exp(-lam * dist)                        # "edge weight" matrix
    Z = E + offdiag(E @ E)                          # softmax denominator
    W = demand / Z    (demand = od offdiag; od > 0 and Z > 0 off-diag
                       for this input family; diag handled by od=0 and
                       Z-diag = round-trip mass > 0)
    flows = W*E + E*(W @ E^T) + E*(E^T @ W)

Sharding with node-relabeling: the computation is equivariant under a
symmetric permutation of nodes, so core i receives all matrices rolled
by -48*i on both axes. Its origin slice is then ALWAYS rows 0..47, and
its flow contribution lands in the tile-0 partitions 0..47 of the p3
output. Host un-rolls the outputs and sums in f32.

Device-side structure (E and E^T are computed on the HOST and shipped
bf16 — no Scalar activations, no activation-table load on device):
    EEs  = identb@Es + Es @ E on the PE; the lhsT for Es @ E is a free
           host-provided column-slice of E^T (est), so no PE transposes
           are needed on the front path.  diag(EEs) is the round-trip
           path mass > 0, od's diag is host-zeroed: no eps needed.
    W    = od (.) reciprocal_approx_fast(EEs)       # 2 DVE ops
    p3   = E (.) (Es^T @ W  +  rows0:48[ W + W @ E^T ])
           Es^T @ W tiles 1,2 ship early on their own queues; the
           tile-0 psum accumulates the T2 terms into partitions 0:48
           (W^T via 3 PE transposes + Scalar psum->sbuf copies) and
           ships last.
All matmul operands bf16 (f32 psum); outputs f16; host sums in f32.
"""

import ml_dtypes
import numpy as np

import concourse.bass as bass
import concourse.mybir as mybir
import concourse.tile as tile
from concourse import bacc
from concourse.bass_utils import run_bass_kernel_spmd
from concourse.masks import make_identity

N = 384
NCORES = 8
S = N // NCORES  # 48 origins per core
P = 128
NT = N // P  # 3 partition tiles

F32 = mybir.dt.float32
F16 = mybir.dt.float16
BF16 = mybir.dt.bfloat16

BF = ml_dtypes.bfloat16


def build_program() -> bass.Bass:
    nc = bacc.Bacc(
        "TRN2",
        target_bir_lowering=False,
        debug=False,
        num_devices=NCORES,
        enable_asserts=False,
    )

    ein_d = nc.dram_tensor("ein", [P, NT, N], BF16, kind="ExternalInput")
    etin_d = nc.dram_tensor("etin", [P, NT, N], BF16, kind="ExternalInput")
    est_d = nc.dram_tensor("est", [P, NT, S], BF16, kind="ExternalInput")
    odt_d = nc.dram_tensor("odt", [S, N], F32, kind="ExternalInput")
    p3_d = nc.dram_tensor("p3_t", [P, NT, N], F16, kind="ExternalOutput")

    with tile.TileContext(nc) as tc:
        with (
            tc.tile_pool(name="sb", bufs=1) as sb,
            tc.tile_pool(name="pst", bufs=3, space="PSUM") as pst,
            tc.tile_pool(name="psacc", bufs=1, space="PSUM") as psacc,
            tc.tile_pool(name="psp3", bufs=1, space="PSUM") as psp3,
        ):
            ein = sb.tile([P, NT, N], BF16)
            etin = sb.tile([P, NT, N], BF16)
            est = sb.tile([P, NT, S], BF16)
            ods = sb.tile([S, N], F32)

            # ---- input DMA: three parallel queues; the front-path
            #      tensors (ein tiles, est, od) land first ----
            nc.gpsimd.dma_start(est[:], est_d[:])
            nc.sync.dma_start(ein[:, 0, :], ein_d[:, 0, :])
            nc.sync.dma_start(ein[:, 1, :], ein_d[:, 1, :])
            nc.sync.dma_start(ein[:, 2, :], ein_d[:, 2, :])
            nc.scalar.dma_start(etin[:], etin_d[:])

            identb = sb.tile([S, S], BF16)
            make_identity(nc, identb[:])
            nc.gpsimd.dma_start(ods[:], odt_d[:])

            Es = ein[0:S, 0, :]  # origin rows 0..47 in rolled space

            # ---- EEs = Es + Es @ E  (Z denominator, f32 psum) ----
            EEs = psacc.tile([S, N], F32, tag="EEs")
            nc.tensor.matmul(EEs[:], identb[:], Es, start=True, stop=False)
            for t in range(NT):
                nc.tensor.matmul(
                    EEs[:], est[:, t, :], ein[:, t, :],
                    start=False, stop=(t == NT - 1),
                )

            # ---- W = od (.) recip(EEs) ----
            zinv = sb.tile([S, N], F32)
            W = sb.tile([S, N], BF16)
            nc.vector.reciprocal_approx_fast(zinv[:], EEs[:])
            nc.vector.tensor_mul(W[:], ods[:], zinv[:])

            out_big = sb.tile([P, NT, N], F16)
            WsT = sb.tile([P, NT, S], BF16)

            # ---- P3 tile 1: ship early on the sync queue ----
            P1 = psp3.tile([P, N], F32, tag="P1")
            nc.tensor.matmul(P1[:], Es[:, P : 2 * P], W[:], start=True, stop=True)
            nc.vector.tensor_mul(out_big[:, 1, :], ein[:, 1, :], P1[:])
            nc.sync.dma_start(p3_d[:, 1, :], out_big[:, 1, :])

            # ---- W^T via PE transposes; copies on the Scalar engine ----
            for c in range(NT):
                tp = pst.tile([P, S], BF16, tag="tp", bufs=3)
                nc.tensor.transpose(tp[:], W[:, P * c : P * (c + 1)], identb[:])
                nc.scalar.copy(WsT[:, c, :], tp[:])

            # ---- P3 tile 2 on the scalar queue ----
            P2 = psp3.tile([P, N], F32, tag="P2")
            nc.tensor.matmul(P2[:], Es[:, 2 * P : N], W[:], start=True, stop=True)
            nc.vector.tensor_mul(out_big[:, 2, :], ein[:, 2, :], P2[:])
            nc.scalar.dma_start(p3_d[:, 2, :], out_big[:, 2, :])

            # ---- P3 tile 0 (+ T2 = W + W @ E^T into rows 0:48), last ----
            P0 = psp3.tile([P, N], F32, tag="P0")
            nc.tensor.matmul(P0[:], Es[:, 0:P], W[:], start=True, stop=False)
            nc.tensor.matmul(P0[0:S, :], identb[:], W[:], start=False, stop=False)
            for c in range(NT):
                nc.tensor.matmul(
                    P0[0:S, :], WsT[:, c, :], etin[:, c, :],
                    start=False, stop=(c == NT - 1),
                )
            nc.vector.tensor_mul(out_big[:, 0, :], ein[:, 0, :], P0[:])
            nc.gpsimd.dma_start(p3_d[:, 0, :], out_big[:, 0, :])

    nc.compile()
    return nc


_PROGRAM_CACHE: dict = {}


def _get_program(lam: float = 0.0) -> bass.Bass:
    # lam only affects host-side marshaling; one program serves all lam
    if "nc" not in _PROGRAM_CACHE:
        _PROGRAM_CACHE["nc"] = build_program()
    return _PROGRAM_CACHE["nc"]


def _tile_rows(x: np.ndarray) -> np.ndarray:
    """[384, N] row-major -> [128, 3, N] partition-tiled layout."""
    return np.ascontiguousarray(x.reshape(NT, P, -1).transpose(1, 0, 2))


def _untile_rows(x: np.ndarray) -> np.ndarray:
    """[128, 3, N] partition-tiled -> [384, N]."""
    return x.transpose(1, 0, 2).reshape(N, -1)


def make_in_maps(od, adj, dist, lam=1.0):
    eye = np.eye(N, dtype=bool)
    A = adj.astype(bool) & ~eye
    E = np.where(A, np.exp(-lam * dist.astype(np.float64)), 0.0).astype(np.float32)
    odz = od.astype(np.float32).copy()
    np.fill_diagonal(odz, 0.0)
    in_maps = []
    for i in range(NCORES):
        r = S * i
        Er = np.roll(E, (-r, -r), axis=(0, 1))
        ein = _tile_rows(Er).astype(BF)
        etin = _tile_rows(np.ascontiguousarray(Er.T)).astype(BF)
        est = np.ascontiguousarray(etin[:, :, 0:S])
        ods = np.ascontiguousarray(np.roll(odz, (-r, -r), axis=(0, 1))[:S])
        in_maps.append({"ein": ein, "etin": etin, "est": est, "odt": ods})
    return in_maps


def gather(results) -> np.ndarray:
    out = np.zeros((N, N), np.float32)
    for i in range(NCORES):
        r = S * i
        p3f = _untile_rows(results[i]["p3_t"]).astype(np.float32)
        out += np.roll(p3f, (r, r), axis=(0, 1))
    return out


def kernel(od, adj, dist, lambda_param, capacity=None, **_unused) -> np.ndarray:
    od = np.ascontiguousarray(np.asarray(od, dtype=np.float32))
    adj = np.ascontiguousarray(np.asarray(adj, dtype=np.int32))
    dist = np.ascontiguousarray(np.asarray(dist, dtype=np.float32))
    lam = float(np.asarray(lambda_param))
    nc = _get_program()
    res = run_bass_kernel_spmd(
        nc, make_in_maps(od, adj, dist, lam), list(range(NCORES))
    )
    return gather(res.results)


# revision 11
# speedup vs baseline: 1.0239x; 1.0239x over previous
"""Trainium2 Bass kernel for the bilevel logit-assignment flow problem.

Reference math (N=384, cutoff-2 paths):
    A = (adj > 0) & ~eye
    E = A * exp(-lam * dist)                        # "edge weight" matrix
    Z = E + offdiag(E @ E)                          # softmax denominator
    W = demand / Z    (demand = od offdiag; od > 0 and Z > 0 off-diag
                       for this input family; diag handled by od=0 and
                       Z-diag = round-trip mass > 0)
    flows = W*E + E*(W @ E^T) + E*(E^T @ W)

Sharding with node-relabeling: the computation is equivariant under a
symmetric permutation of nodes, so core i receives all matrices rolled
by -48*i on both axes. Its origin slice is then ALWAYS rows 0..47, and
its flow contribution lands in the tile-0 partitions 0..47 of the p3
output. Host un-rolls the outputs and sums in f32.

Device-side structure (E and E^T are computed on the HOST and shipped
bf16 — no Scalar exp activations on device):
    estid   = [EsT tiles | identity] — the lhsT for Es @ E is a free
              host-provided column-slice of E^T, and the 48x48 identity
              rides along in slot 3, so nothing is built on GpSimd.
    EEs     = identb@Es + Es @ E on the PE (f32 psum); diag(EEs) is the
              round-trip path mass > 0, od's diag host-zeroed: no eps.
    W       = od (.) reciprocal_approx_fast(EEs), produced in column
              halves so P3 tile 1 can start before the second half.
    p3      = E (.) (Es^T @ W  +  rows0:48[ W + W @ E^T ])
              tiles 1, 2 and tile-0 partitions 48:128 ship early; the
              48-partition T2 slice ships last (smallest tail DMA).
    Warm-up matmuls on zeros keep the PE busy from ~1.5us so the HAM
    clock-gate reaches 2.4 GHz before the post-W matmul burst.
Inputs stream on the two hardware DGE queues (SP + Activation) —
the GpSimd software queue is an order of magnitude slower and is not
used. All matmul operands bf16 (f32 psum); outputs f16; host sums f32.
"""

import ml_dtypes
import numpy as np

import concourse.bass as bass
import concourse.mybir as mybir
import concourse.tile as tile
from concourse import bacc
from concourse.bass_utils import run_bass_kernel_spmd

N = 384
NCORES = 8
S = N // NCORES  # 48 origins per core
P = 128
NT = N // P  # 3 partition tiles
H = N // 2

F32 = mybir.dt.float32
F16 = mybir.dt.float16
BF16 = mybir.dt.bfloat16

BF = ml_dtypes.bfloat16

NWARM = 4  # PE warm-up matmuls (N=384 each) before real work
NSTALL = 2  # short (N=128) fillers during the W dependency stall


def build_program() -> bass.Bass:
    nc = bacc.Bacc(
        "TRN2",
        target_bir_lowering=False,
        debug=False,
        num_devices=NCORES,
        enable_asserts=False,
    )

    ein_d = nc.dram_tensor("ein", [P, NT, N], BF16, kind="ExternalInput")
    etin_d = nc.dram_tensor("etin", [P, NT, N], BF16, kind="ExternalInput")
    estid_d = nc.dram_tensor("estid", [P, NT + 1, S], BF16, kind="ExternalInput")
    odt_d = nc.dram_tensor("odt", [S, N], BF16, kind="ExternalInput")
    p3_d = nc.dram_tensor("p3_t", [P, NT, N], F16, kind="ExternalOutput")

    with tile.TileContext(nc) as tc:
        with (
            tc.tile_pool(name="sb", bufs=1) as sb,
            tc.tile_pool(name="pst", bufs=3, space="PSUM") as pst,
            tc.tile_pool(name="psacc", bufs=1, space="PSUM") as psacc,
            tc.tile_pool(name="psp3", bufs=1, space="PSUM") as psp3,
        ):
            ein = sb.tile([P, NT, N], BF16)
            etin = sb.tile([P, NT, N], BF16)
            estid = sb.tile([P, NT + 1, S], BF16)
            ods = sb.tile([S, N], BF16)
            junkz = sb.tile([S, N], BF16)

            # ---- input DMA: balanced across the two HW DGE queues, in
            #      need-by order (front-path tensors first) ----
            nc.sync.dma_start(estid[:], estid_d[:])
            nc.sync.dma_start(ein[:, 0, :], ein_d[:, 0, :])
            nc.sync.dma_start(etin[:, 0, :], etin_d[:, 0, :])
            nc.sync.dma_start(etin[:, 1, :], etin_d[:, 1, :])
            nc.scalar.dma_start(ein[:, 1, :], ein_d[:, 1, :])
            nc.scalar.dma_start(ein[:, 2, :], ein_d[:, 2, :])
            nc.scalar.dma_start(ods[:], odt_d[:])
            nc.scalar.dma_start(etin[:, 2, :], etin_d[:, 2, :])

            identb = estid[0:S, NT, :]
            Es = ein[0:S, 0, :]  # origin rows 0..47 in rolled space

            # ---- PE warm-up on zeros (HAM clock ramp); scratch psum is
            #      the not-yet-used P1/P2 tiles (WAW-ordered by Tile) ----
            nc.vector.memset(junkz[:], 0.0)
            P1 = psp3.tile([P, N], F32, tag="P1")
            P2 = psp3.tile([P, N], F32, tag="P2")
            for _ in range(NWARM):
                nc.tensor.matmul(
                    P1[0:S, :], junkz[:, 0:S], junkz[:], start=True, stop=True
                )

            # ---- EEs = Es + Es @ E, accumulated in DMA-arrival order ----
            EEs = psacc.tile([S, N], F32, tag="EEs")
            nc.tensor.matmul(
                EEs[:], estid[:, 1, :], ein[:, 1, :], start=True, stop=False
            )
            nc.tensor.matmul(EEs[:], identb, Es, start=False, stop=False)
            nc.tensor.matmul(
                EEs[:], estid[:, 0, :], ein[:, 0, :], start=False, stop=False
            )
            nc.tensor.matmul(
                EEs[:], estid[:, 2, :], ein[:, 2, :], start=False, stop=True
            )
            for _ in range(NSTALL):
                nc.tensor.matmul(
                    P2[0:S, 0:P], junkz[:, 0:S], junkz[:, 0:P],
                    start=True, stop=True,
                )

            # ---- W = od (.) recip(EEs), in column halves ----
            zinv = sb.tile([S, N], F32)
            W = sb.tile([S, N], BF16)
            nc.vector.reciprocal_approx_fast(zinv[:, 0:H], EEs[:, 0:H])
            nc.vector.tensor_mul(W[:, 0:H], ods[:, 0:H], zinv[:, 0:H])
            nc.vector.reciprocal_approx_fast(zinv[:, H:N], EEs[:, H:N])
            nc.vector.tensor_mul(W[:, H:N], ods[:, H:N], zinv[:, H:N])

            out_big = sb.tile([P, NT, N], F16)
            WsT = sb.tile([P, NT, S], BF16)

            # ---- P3 tile 1 (split to start on the first W half) ----
            nc.tensor.matmul(
                P1[:, 0:H], Es[:, P : 2 * P], W[:, 0:H], start=True, stop=True
            )
            tp0 = pst.tile([P, S], BF16, tag="tp", bufs=3)
            nc.tensor.transpose(tp0[:], W[:, 0:P], identb)
            nc.scalar.copy(WsT[:, 0, :], tp0[:])
            nc.tensor.matmul(
                P1[:, H:N], Es[:, P : 2 * P], W[:, H:N], start=True, stop=True
            )
            nc.vector.tensor_mul(out_big[:, 1, :], ein[:, 1, :], P1[:])
            nc.sync.dma_start(p3_d[:, 1, :], out_big[:, 1, :])

            # ---- remaining W^T chunks ----
            for c in range(1, NT):
                tp = pst.tile([P, S], BF16, tag="tp", bufs=3)
                nc.tensor.transpose(tp[:], W[:, P * c : P * (c + 1)], identb)
                nc.scalar.copy(WsT[:, c, :], tp[:])

            # ---- P3 tile 2 ----
            nc.tensor.matmul(P2[:], Es[:, 2 * P : N], W[:], start=True, stop=True)
            nc.vector.tensor_mul(out_big[:, 2, :], ein[:, 2, :], P2[:])
            nc.scalar.dma_start(p3_d[:, 2, :], out_big[:, 2, :])

            # ---- P3 tile 0, partitions 64:128 (no T2 terms: ship early;
            #      DVE partition offsets must be multiples of 32) ----
            HP = P // 2
            P0hi = psp3.tile([HP, N], F32, tag="P0hi")
            nc.tensor.matmul(P0hi[:], Es[:, HP:P], W[:], start=True, stop=True)
            nc.vector.tensor_mul(out_big[HP:P, 0, :], ein[HP:P, 0, :], P0hi[:])
            nc.sync.dma_start(p3_d[HP:P, 0, :], out_big[HP:P, 0, :])

            # ---- P3 tile 0, partitions 0:64 + T2 = W + W @ E^T into the
            #      first 48 partitions, shipped last ----
            P0lo = psp3.tile([HP, N], F32, tag="P0lo")
            nc.tensor.matmul(P0lo[:], Es[:, 0:HP], W[:], start=True, stop=False)
            nc.tensor.matmul(P0lo[0:S, :], identb, W[:], start=False, stop=False)
            for c in range(NT):
                nc.tensor.matmul(
                    P0lo[0:S, :], WsT[:, c, :], etin[:, c, :],
                    start=False, stop=(c == NT - 1),
                )
            nc.vector.tensor_mul(out_big[0:HP, 0, :], ein[0:HP, 0, :], P0lo[:])
            nc.sync.dma_start(p3_d[0:HP, 0, :], out_big[0:HP, 0, :])

    nc.compile()
    return nc


_PROGRAM_CACHE: dict = {}


def _get_program(lam: float = 0.0) -> bass.Bass:
    # lam only affects host-side marshaling; one program serves all lam
    if "nc" not in _PROGRAM_CACHE:
        _PROGRAM_CACHE["nc"] = build_program()
    return _PROGRAM_CACHE["nc"]


def _tile_rows(x: np.ndarray) -> np.ndarray:
    """[384, N] row-major -> [128, 3, N] partition-tiled layout."""
    return np.ascontiguousarray(x.reshape(NT, P, -1).transpose(1, 0, 2))


def _untile_rows(x: np.ndarray) -> np.ndarray:
    """[128, 3, N] partition-tiled -> [384, N]."""
    return x.transpose(1, 0, 2).reshape(N, -1)


def make_in_maps(od, adj, dist, lam=1.0):
    eye = np.eye(N, dtype=bool)
    A = adj.astype(bool) & ~eye
    E = np.where(A, np.exp(-lam * dist.astype(np.float64)), 0.0).astype(np.float32)
    odz = od.astype(np.float32).copy()
    np.fill_diagonal(odz, 0.0)
    ident = np.zeros((P, 1, S), np.float32)
    ident[0:S, 0, :] = np.eye(S, dtype=np.float32)
    in_maps = []
    for i in range(NCORES):
        r = S * i
        Er = np.roll(E, (-r, -r), axis=(0, 1))
        ein = _tile_rows(Er).astype(BF)
        etin = _tile_rows(np.ascontiguousarray(Er.T)).astype(BF)
        estid = np.ascontiguousarray(
            np.concatenate([etin[:, :, 0:S], ident.astype(BF)], axis=1)
        )
        ods = np.ascontiguousarray(
            np.roll(odz, (-r, -r), axis=(0, 1))[:S]
        ).astype(BF)
        in_maps.append({"ein": ein, "etin": etin, "estid": estid, "odt": ods})
    return in_maps


def gather(results) -> np.ndarray:
    out = np.zeros((N, N), np.float32)
    for i in range(NCORES):
        r = S * i
        p3f = _untile_rows(results[i]["p3_t"]).astype(np.float32)
        out += np.roll(p3f, (r, r), axis=(0, 1))
    return out


def kernel(od, adj, dist, lambda_param, capacity=None, **_unused) -> np.ndarray:
    od = np.ascontiguousarray(np.asarray(od, dtype=np.float32))
    adj = np.ascontiguousarray(np.asarray(adj, dtype=np.int32))
    dist = np.ascontiguousarray(np.asarray(dist, dtype=np.float32))
    lam = float(np.asarray(lambda_param))
    nc = _get_program()
    res = run_bass_kernel_spmd(
        nc, make_in_maps(od, adj, dist, lam), list(range(NCORES))
    )
    return gather(res.results)
